# revision 5
# baseline (speedup 1.0000x reference)
"""Trainium2 Bass kernel for streaming dot-product attention with alpha decay.

Math restructure: with e~_s = alpha^{-s} * exp(qk_s) (the QK_max shift and the
alpha^t decay cancel in the ratio QKV_t / Z_t), the scan
  QKV_t = a*QKV_{t-1} + e_t (x) v_t ;  Z_t = a*Z_{t-1} + e_t ;  out_t = QKV_t/Z_t
becomes a pure prefix sum:
  out_t = (QKV_0 + sum_{s<=t} e~_s (x) v_s) / (Z_0 + sum_{s<=t} e~_s)
which maps onto the TensorEngine as a triangular-ones matmul over the stream
axis.  The device returns the four pieces (QKV_0, Z_0, stream-prefix numerator,
stream-prefix denominator) in fp16; the host adds the init terms and performs
the division in fp32 (gather/unshard and input marshalling are host-side by
contract).  q/k tensors arrive pre-transposed from the host so the d-axis is
already on partitions; v_init carries a baked all-ones 65th column so the
QKV_0 matmul also produces Z_0.  All matmuls run fp16 with fp32 PSUM.
Each core handles 8 of the 64 batch rows (B sharded across 8 cores).

Engine budget per core (target ~40us): Vector = R-build cols 0:36 + 24-28
PSUM evictions; GpSimd = R-build cols 36:64; Scalar = exps + den copies +
36-40 PSUM evictions; PE = 18 matmuls/b; one 1MB output DMA per b.
"""

import math
from contextlib import ExitStack

import numpy as np

import concourse.bass as bass
import concourse.bacc as bacc
import concourse.tile as tile
from concourse import mybir
from concourse.bass_utils import run_bass_kernel_spmd

ALPHA = 0.99
B, N1, N2, D, T = 64, 64, 512, 64, 128
NCORES = 8
BL = B // NCORES  # batch rows per core
NCH = 8           # n-chunks per b; each chunk covers 8 n values = 512 psum cols
RSPLIT = 36       # R-build: vector does n cols [0,RSPLIT), gpsimd the rest
F32 = mybir.dt.float32
F16 = mybir.dt.float16
Exp = mybir.ActivationFunctionType.Exp


def _build():
    nc = bacc.Bacc("TRN2", target_bir_lowering=False, debug=False)

    qT_d = nc.dram_tensor("qT", [D, BL, N1], F16, kind="ExternalInput")
    kT_d = nc.dram_tensor("kT", [D, BL, N2], F16, kind="ExternalInput")
    vin_d = nc.dram_tensor("vinp", [128, BL, 4, D + 1], F16, kind="ExternalInput")
    ksT_d = nc.dram_tensor("ksT", [D, BL, T], F16, kind="ExternalInput")
    vst_d = nc.dram_tensor("v_stream", [T, BL, D], F16, kind="ExternalInput")
    tri_d = nc.dram_tensor("tri", [T, T], F16, kind="ExternalInput")
    sb_d = nc.dram_tensor("sbias", [T, 1], F32, kind="ExternalInput")
    nout_d = nc.dram_tensor("nout", [T, BL, N1, D], F16, kind="ExternalOutput")
    dout_d = nc.dram_tensor("dout", [T, BL, N1], F16, kind="ExternalOutput")
    n0_d = nc.dram_tensor("n0", [N1, BL, D], F16, kind="ExternalOutput")
    d0_d = nc.dram_tensor("d0", [N1, BL], F16, kind="ExternalOutput")

    with tile.TileContext(nc) as tc, ExitStack() as ctx:
        consts = ctx.enter_context(tc.tile_pool(name="consts", bufs=1))
        inbuf = ctx.enter_context(tc.tile_pool(name="inbuf", bufs=1))
        small = ctx.enter_context(tc.tile_pool(name="small", bufs=4))
        rbuf = ctx.enter_context(tc.tile_pool(name="rbuf", bufs=4))
        obuf = ctx.enter_context(tc.tile_pool(name="obuf", bufs=4))
        accum = ctx.enter_context(tc.tile_pool(name="accum", bufs=1))
        psum = ctx.enter_context(tc.tile_pool(name="psum", bufs=1, space="PSUM"))

        tri = consts.tile([T, T], F16)
        nc.sync.dma_start(out=tri[:], in_=tri_d[:])
        sbias = consts.tile([T, 1], F32)
        nc.sync.dma_start(out=sbias[:], in_=sb_d[:])

        qT_all = inbuf.tile([D, BL, N1], F16)
        kT_all = inbuf.tile([D, BL, N2], F16)
        vin_all = inbuf.tile([128, BL, 4, D + 1], F16)
        ksT_all = inbuf.tile([D, BL, T], F16)
        vst_all = inbuf.tile([T, BL, D], F16)
        nc.sync.dma_start(out=qT_all[:], in_=qT_d[:])
        nc.scalar.dma_start(out=kT_all[:], in_=kT_d[:])
        nc.scalar.dma_start(out=vin_all[:], in_=vin_d[:])
        nc.sync.dma_start(out=ksT_all[:], in_=ksT_d[:])
        nc.sync.dma_start(out=vst_all[:], in_=vst_d[:])

        # accumulation tiles for the small outputs (one DMA each at the end)
        den_all = accum.tile([T, BL, N1], F16)
        n0_sb = accum.tile([N1, BL, D], F16)
        d0_sb = accum.tile([N1, BL], F16)

        # --- phase 0: all init attention + stream exps, dense on PE/ACT ---
        ebs = []
        for b in range(BL):
            # stream logits first: eb is on the R-build critical path
            ps_s = psum.tile([T, N1], F32, tag="pa", bufs=3)
            nc.tensor.matmul(
                ps_s[:], ksT_all[:, b, :], qT_all[:, b, :], start=True, stop=True
            )
            eb = small.tile([T, N1], F16, tag="eb", bufs=8)
            nc.scalar.activation(eb[:], ps_s[:], Exp, bias=sbias[:], scale=1.0)
            ebs.append(eb)

            qk_ps = psum.tile([128, 4, N1], F32, tag="pa", bufs=3)
            for c in range(4):
                nc.tensor.matmul(
                    qk_ps[:, c, :], kT_all[:, b, 128 * c : 128 * (c + 1)],
                    qT_all[:, b, :], start=True, stop=True,
                )
            qke = small.tile([128, 4, N1], F16, tag="qke")
            nc.scalar.activation(qke[:], qk_ps[:], Exp)

            # [QKV_0 | Z_0] in one bank: cols 0..63 = QKV_0[n, d], col 64 = Z_0
            p0 = psum.tile([N1, D + 1], F32, tag="p0", bufs=2)
            for c in range(4):
                nc.tensor.matmul(
                    p0[:], qke[:, c, :], vin_all[:, b, c, :],
                    start=(c == 0), stop=(c == 3),
                )
            nc.vector.tensor_copy(n0_sb[:, b, :], p0[:, 0:D])
            nc.vector.tensor_copy(d0_sb[:, b : b + 1], p0[:, D : D + 1])

        # --- phase 1 (pipelined over b): R-build -> prefix matmuls -> evict ---
        for b in range(BL):
            eb = ebs[b]
            # R[s, n, d] = e~[s, n] * v[s, d]  (fp16), split across DVE/GpSimd
            R_t = rbuf.tile([T, N1, D], F16, tag="R")
            nc.vector.tensor_mul(
                R_t[:, 0:RSPLIT, :],
                eb[:, 0:RSPLIT, None].broadcast_to([T, RSPLIT, D]),
                vst_all[:, b, None, :].broadcast_to([T, RSPLIT, D]),
            )
            nc.gpsimd.tensor_mul(
                R_t[:, RSPLIT:N1, :],
                eb[:, RSPLIT:N1, None].broadcast_to([T, N1 - RSPLIT, D]),
                vst_all[:, b, None, :].broadcast_to([T, N1 - RSPLIT, D]),
            )

            # den[t, n] = sum_{s<=t} e~[s, n]   (Z_0 added on host)
            pden = psum.tile([T, N1], F32, tag="pa", bufs=3)
            nc.tensor.matmul(pden[:], tri[:], eb[:], start=True, stop=True)
            nc.scalar.copy(den_all[:, b, :], pden[:])

            # num chunks -> fp16 staging tile, one output DMA per b
            o_sb = obuf.tile([T, N1, D], F16, tag="osb")
            for c in range(NCH):
                pnum = psum.tile([T, 8, D], F32, tag="pbig", bufs=3)
                nc.tensor.matmul(
                    pnum[:], tri[:], R_t[:, 8 * c : 8 * (c + 1), :],
                    start=True, stop=True,
                )
                if c < 2:
                    nc.vector.tensor_copy(o_sb[:, 8 * c : 8 * (c + 1), :], pnum[:])
                else:
                    nc.scalar.copy(o_sb[:, 8 * c : 8 * (c + 1), :], pnum[:])
            eng = nc.sync if b % 2 == 0 else nc.scalar
            eng.dma_start(out=nout_d[:, b], in_=o_sb[:])

        nc.sync.dma_start(out=dout_d[:], in_=den_all[:])
        nc.scalar.dma_start(out=n0_d[:], in_=n0_sb[:])
        nc.sync.dma_start(out=d0_d[:], in_=d0_sb[:])

    nc.compile()
    return nc


_CACHE = {}


def _get_nc():
    if "nc" not in _CACHE:
        _CACHE["nc"] = _build()
    return _CACHE["nc"]


def _in_maps(q, k_init, v_init, k_stream, v_stream):
    q = np.asarray(q, np.float32).astype(np.float16)
    k_init = np.asarray(k_init, np.float32).astype(np.float16)
    v_init = np.asarray(v_init, np.float32).astype(np.float16)
    k_stream = np.asarray(k_stream, np.float32).astype(np.float16)
    v_stream = np.asarray(v_stream, np.float32).astype(np.float16)
    qT = q.transpose(2, 0, 1)                      # [D, B, N1]
    kT = k_init.transpose(2, 0, 1)                 # [D, B, N2]
    ksT = k_stream.transpose(2, 1, 0)              # [D, B, T]
    # v_init with baked ones column, chunked m = 128*c + p
    vinp = np.ones((B, 4, 128, D + 1), np.float16)
    vinp[:, :, :, 0:D] = v_init.reshape(B, 4, 128, D)
    vinp = vinp.transpose(2, 0, 1, 3)              # [128, B, 4, D+1]
    tri = np.triu(np.ones((T, T), np.float32)).astype(np.float16)
    sbias = (np.arange(1, T + 1, dtype=np.float64) * (-math.log(ALPHA))).astype(
        np.float32
    ).reshape(T, 1)
    maps = []
    for i in range(NCORES):
        sl = slice(i * BL, (i + 1) * BL)
        maps.append(
            dict(
                qT=np.ascontiguousarray(qT[:, sl]),
                kT=np.ascontiguousarray(kT[:, sl]),
                vinp=np.ascontiguousarray(vinp[:, sl]),
                ksT=np.ascontiguousarray(ksT[:, sl]),
                v_stream=np.ascontiguousarray(v_stream[:, sl]),
                tri=tri,
                sbias=sbias,
            )
        )
    return maps


def run(q, k_init, v_init, attn_mask, k_stream, v_stream, trace=False, **trace_kw):
    """Run on hardware; returns (output, BassKernelResults)."""
    nc = _get_nc()
    maps = _in_maps(q, k_init, v_init, k_stream, v_stream)
    res = run_bass_kernel_spmd(nc, maps, list(range(NCORES)), trace=trace, **trace_kw)
    out = np.empty((T + 1, B, N1, D), np.float32)
    for i in range(NCORES):
        r = res.results[i]
        n0 = np.asarray(r["n0"], np.float32).transpose(1, 0, 2)   # [BL, N1, D]
        d0 = np.asarray(r["d0"], np.float32).T                    # [BL, N1]
        ns = np.asarray(r["nout"], np.float32)                    # [T, BL, N1, D]
        ds = np.asarray(r["dout"], np.float32)                    # [T, BL, N1]
        sl = slice(i * BL, (i + 1) * BL)
        out[0, sl] = n0 / d0[..., None]
        out[1:, sl] = (n0[None] + ns) / (d0[None] + ds)[..., None]
    return out, res


def kernel(q, k_init, v_init, attn_mask, k_stream, v_stream):
    out, _ = run(q, k_init, v_init, attn_mask, k_stream, v_stream, trace=False)
    return out
